# revision 13
# baseline (speedup 1.0000x reference)
"""Trainium2 Bass kernel for nn_GuidedCorrespondenceLoss.

Strategy (8 NeuronCores, SPMD):
  - Host: nearest-neighbor field -> patch indices; gather 7x7x128 patch
    columns into channel-major bf16 matrices; compute Gram-matrix correction
    vectors (means/norms/cross terms) so the device works on RAW gathered
    patches (no centering/normalization pass on device).
  - Device (per core, target rows sharded 512/core): S = Xg^T Yg via bf16
    matmuls (K=6272 on partitions, 49 k-tiles), mask-patch L2 matmul (K=49,
    f32), fused distance combine on Vector/Scalar engines, then row-min and
    exp-sum row stats (relaxed contextual loss).
  - Host: loss_i = log(sumw_i) - 2*(1 - dmin_i/(dmin_i+EPS)); mean over 4096.

The reference l2_distance has an index quirk (verified numerically):
  d_prog[i,j] = clip(||tp_i||^2 + ||rp_j||^2 - 2*tp[:,j].rp[:,i], 0)/49
so the device L2 matmul uses lhsT = rp[:, target_rows], rhs = tp[:, :].
"""
import numpy as np
import ml_dtypes

EPS = 2.220446049250313e-16
PATCH, STRIDE, SAMPLE, H_BW, PROG_W = 7, 3, 64, 0.5, 10.0
HO = (256 - PATCH) // STRIDE + 1     # 84

N_CORES = 8
NT = 4096            # total target rows
TPC = NT // N_CORES  # 512 target rows per core
MT = TPC // 128      # 4 m-tiles per core
NR = 4096            # refer columns
BLK = 256            # columns per block
NB = NR // BLK       # 16 blocks
K = PATCH * PATCH    # 49 contraction tiles of 128 channels
CH = 128

_PROGRAM_CACHE = {}


def _field_to_idx(field):
    g = np.asarray(field)[0].reshape(-1, 2).astype(np.float32)
    gx = (g[:, 0] + np.float32(1.0)) * np.float32(0.5) * np.float32(HO - 1)
    gy = (g[:, 1] + np.float32(1.0)) * np.float32(0.5) * np.float32(HO - 1)
    ix = np.clip(np.round(gx), 0, HO - 1).astype(np.int64)
    iy = np.clip(np.round(gy), 0, HO - 1).astype(np.int64)
    return iy, ix


def _gather_cols(feat_chw, iy, ix):
    """feat [C,256,256] -> [C, 49, n]: out[c,k,j] = feat[c, 3*iy+kh, 3*ix+kw]."""
    iy3, ix3 = iy * STRIDE, ix * STRIDE
    kh = np.repeat(np.arange(PATCH), PATCH)
    kw = np.tile(np.arange(PATCH), PATCH)
    HH = iy3[None, :] + kh[:, None]
    WW = ix3[None, :] + kw[:, None]
    return feat_chw[:, HH, WW]


def _build_kernel_body(tc, aps):
    import concourse.bass as bass
    from concourse import mybir

    nc = tc.nc
    f32 = mybir.dt.float32
    bf16 = mybir.dt.bfloat16
    AF = mybir.ActivationFunctionType
    OP = mybir.AluOpType

    xk, yk, rowt, colt, mpk, stats = (
        aps["xk"], aps["yk"], aps["rowt"], aps["colt"],
        aps["mpk"], aps["stats"],
    )

    YH0 = 25 * BLK               # first-half cols (k = 0..24)
    YH1 = (K - 25) * BLK         # second-half cols (k = 25..48)

    with (
        tc.tile_pool(name="xpool", bufs=1) as xpool,
        tc.tile_pool(name="ypool", bufs=3) as ypool,
        tc.tile_pool(name="dpool", bufs=1) as dpool,
        tc.tile_pool(name="cpool", bufs=2) as cpool,
        tc.tile_pool(name="mpool", bufs=1) as mpool,
        tc.tile_pool(name="tpool", bufs=2) as tpool,
        tc.tile_pool(name="spool", bufs=8) as spool,
        tc.tile_pool(name="psS", bufs=4, space="PSUM") as psS,
        tc.tile_pool(name="psP", bufs=2, space="PSUM") as psP,
    ):
        # ---- resident loads ------------------------------------------------
        x_t = xpool.tile([CH, K * TPC], bf16, name="x_t")
        nc.sync.dma_start(x_t[:], xk[:])

        rowt_t = xpool.tile([CH, 4 * MT], f32, name="rowt_t")
        nc.sync.dma_start(rowt_t[:], rowt[:])

        cst = xpool.tile([CH, 2], f32, name="cst")
        nc.vector.memset(cst[:, 0:1], 0.5)
        nc.vector.memset(cst[:, 1:2], 2.0)

        # mq (rp[:,rows], TPC cols) and mk (tp, NR cols) share one DMA/tensor
        mpk_t = mpool.tile([CH, TPC + NR], f32, name="mpk_t")
        nc.sync.dma_start(mpk_t[:K, :], mpk[:])
        mq_t = mpk_t[:, :TPC]
        mk_t = mpk_t[:, TPC:]

        d_t = dpool.tile([CH, MT * NR], f32, name="d_t")

        # ---- main loop -----------------------------------------------------
        for b in range(NB):
            y0 = ypool.tile([CH, YH0], bf16, name=f"y0_{b}", tag="y")
            nc.sync.dma_start(y0[:], yk[:, b * K * BLK: b * K * BLK + YH0])
            y1 = ypool.tile([CH, YH0], bf16, name=f"y1_{b}", tag="y")
            nc.sync.dma_start(
                y1[:, :YH1], yk[:, b * K * BLK + YH0: (b + 1) * K * BLK])

            colt_t = cpool.tile([CH, 3 * BLK], f32, name=f"colt_{b}", tag="c")
            nc.sync.dma_start(colt_t[:], colt[:, b * 3 * BLK: (b + 1) * 3 * BLK])

            mkb = mk_t[:K, b * BLK: (b + 1) * BLK]

            for m in range(MT):
                pp = psP.tile([CH, BLK], f32, name=f"pp_{b}_{m}", tag="pp")
                nc.tensor.matmul(
                    pp[:],
                    lhsT=mq_t[:K, m * 128: (m + 1) * 128],
                    rhs=mkb,
                    start=True, stop=True,
                )
                ps = psS.tile([CH, BLK], f32, name=f"ps_{b}_{m}", tag="ps")
                for k in range(K):
                    if k < 25:
                        rhs = y0[:, k * BLK: (k + 1) * BLK]
                    else:
                        rhs = y1[:, (k - 25) * BLK: (k - 24) * BLK]
                    nc.tensor.matmul(
                        ps[:],
                        lhsT=x_t[:, k * TPC + m * 128: k * TPC + (m + 1) * 128],
                        rhs=rhs,
                        start=(k == 0),
                        stop=(k == K - 1),
                    )

                # sim = (S + a_p + ccol_j) * rxinv_p * cyinv_j
                t2 = tpool.tile([CH, BLK], f32, name=f"t2_{b}_{m}", tag="t2")
                nc.vector.scalar_tensor_tensor(
                    t2[:], in0=ps[:], scalar=rowt_t[:, 4 * m: 4 * m + 1],
                    in1=colt_t[:, 0:BLK], op0=OP.add, op1=OP.add)
                t4 = tpool.tile([CH, BLK], f32, name=f"t4_{b}_{m}", tag="t4")
                nc.vector.scalar_tensor_tensor(
                    t4[:], in0=t2[:], scalar=rowt_t[:, 4 * m + 1: 4 * m + 2],
                    in1=colt_t[:, BLK: 2 * BLK], op0=OP.mult, op1=OP.mult)
                # d_cos = relu(0.5 - 0.5*sim)
                dcos = tpool.tile([CH, BLK], f32, name=f"dcos_{b}_{m}", tag="dcos")
                nc.scalar.activation(dcos[:], t4[:], AF.Relu,
                                     bias=cst[:, 0:1], scale=-0.5)
                # v = relu(tpn_p + rpn_j - 2*P)
                u = tpool.tile([CH, BLK], f32, name=f"u_{b}_{m}", tag="u")
                nc.vector.scalar_tensor_tensor(
                    u[:], in0=pp[:], scalar=-2.0,
                    in1=colt_t[:, 2 * BLK: 3 * BLK], op0=OP.mult, op1=OP.add)
                v = tpool.tile([CH, BLK], f32, name=f"v_{b}_{m}", tag="v")
                nc.scalar.activation(
                    v[:], u[:], AF.Relu,
                    bias=rowt_t[:, 4 * m + 2: 4 * m + 3], scale=1.0)
                # d = dcos + (10/49) * v   -> f32 store
                nc.vector.scalar_tensor_tensor(
                    d_t[:, m * NR + b * BLK: m * NR + (b + 1) * BLK],
                    in0=v[:], scalar=float(PROG_W / K), in1=dcos[:],
                    op0=OP.mult, op1=OP.add)

        # ---- row stats -----------------------------------------------------
        souts = xpool.tile([CH, 2 * MT], f32, name="souts")
        for m in range(MT):
            dm = d_t[:, m * NR: (m + 1) * NR]
            dmin = spool.tile([CH, 1], f32, name=f"dmin_{m}", tag="st")
            nc.vector.tensor_reduce(dmin[:], dm, axis=mybir.AxisListType.X,
                                    op=OP.min)
            dme = spool.tile([CH, 1], f32, name=f"dme_{m}", tag="st")
            nc.vector.tensor_scalar_add(dme[:], dmin[:], float(EPS))
            rec = spool.tile([CH, 1], f32, name=f"rec_{m}", tag="st")
            nc.vector.reciprocal(rec[:], dme[:])
            scl = spool.tile([CH, 1], f32, name=f"scl_{m}", tag="st")
            nc.vector.tensor_scalar_mul(scl[:], rec[:], -2.0)
            # w = exp(2 - 2*d*rec); sumw = sum_j w
            wtmp = ypool.tile([CH, NR], bf16, name=f"wtmp_{m}", tag="y")
            sumw = spool.tile([CH, 1], f32, name=f"sumw_{m}", tag="st")
            nc.scalar.activation(wtmp[:], dm, AF.Exp,
                                 bias=cst[:, 1:2], scale=scl[:, 0:1],
                                 accum_out=sumw[:])
            nc.vector.tensor_copy(souts[:, 2 * m: 2 * m + 1], dmin[:])
            nc.vector.tensor_copy(souts[:, 2 * m + 1: 2 * m + 2], sumw[:])

        nc.sync.dma_start(stats[:], souts[:])


def build_program():
    """Build the SPMD Bass program once. Returns (nc, names)."""
    if "prog" in _PROGRAM_CACHE:
        return _PROGRAM_CACHE["prog"]

    import concourse.tile as tile
    from concourse import bacc, mybir

    f32 = mybir.dt.float32
    bf16 = mybir.dt.bfloat16

    nc = bacc.Bacc("TRN2", target_bir_lowering=False, debug=False,
                   enable_asserts=False, num_devices=N_CORES)
    aps = {
        "xk": nc.dram_tensor("xk", [CH, K * TPC], bf16,
                             kind="ExternalInput").ap(),
        "yk": nc.dram_tensor("yk", [CH, NB * K * BLK], bf16,
                             kind="ExternalInput").ap(),
        "rowt": nc.dram_tensor("rowt", [CH, 4 * MT], f32,
                               kind="ExternalInput").ap(),
        "colt": nc.dram_tensor("colt", [CH, NB * 3 * BLK], f32,
                               kind="ExternalInput").ap(),
        "mpk": nc.dram_tensor("mpk", [K, TPC + NR], f32,
                              kind="ExternalInput").ap(),
        "stats": nc.dram_tensor("stats", [CH, 2 * MT], f32,
                                kind="ExternalOutput").ap(),
    }
    with tile.TileContext(nc) as tc:
        _build_kernel_body(tc, aps)
    nc.compile()

    _PROGRAM_CACHE["prog"] = (nc, aps)
    return nc, aps


def host_prepare(target_features, refer_features, mask, target_field,
                 refer_field):
    """Compute everything the device needs. Returns (in_maps, meta)."""
    tgt = np.asarray(target_features)[0]
    ref = np.asarray(refer_features)[0]
    msk = np.asarray(mask)[0, 0]
    t_iy, t_ix = _field_to_idx(target_field)
    r_iy, r_ix = _field_to_idx(refer_field)

    tgt32 = tgt.astype(ml_dtypes.bfloat16).astype(np.float32)
    ref32 = ref.astype(ml_dtypes.bfloat16).astype(np.float32)

    xg = _gather_cols(tgt32, t_iy, t_ix)       # [128, 49, 4096] f32
    yg = _gather_cols(ref32, r_iy, r_ix)

    xg2 = xg.reshape(CH * K, NT).astype(np.float64)
    yg2 = yg.reshape(CH * K, NR)
    yg64 = yg2.astype(np.float64)
    mu = yg64.mean(axis=1)
    crx = mu @ xg2
    cry = mu @ yg64
    musq = float(mu @ mu)
    xnormsq = np.einsum('ij,ij->j', xg2, xg2) - 2 * crx + musq
    ynormsq = np.einsum('ij,ij->j', yg64, yg64) - 2 * cry + musq
    rxinv = 1.0 / (np.sqrt(np.maximum(xnormsq, 0)) + EPS)
    cyinv = 1.0 / (np.sqrt(np.maximum(ynormsq, 0)) + EPS)
    a_row = musq - crx
    ccol = -cry

    tp = _gather_cols(msk[None], t_iy, t_ix)[0].astype(np.float32)  # [49, NT]
    rp = _gather_cols(msk[None], r_iy, r_ix)[0].astype(np.float32)  # [49, NR]
    tpn = (tp.astype(np.float64) ** 2).sum(0)
    rpn = (rp.astype(np.float64) ** 2).sum(0)

    # device input assembly
    xgb = xg.astype(ml_dtypes.bfloat16)        # [128, 49, 4096]
    ygb = yg.astype(ml_dtypes.bfloat16)
    # yk layout: col = b*K*BLK + k*BLK + j
    yk_arr = np.ascontiguousarray(
        ygb.reshape(CH, K, NB, BLK).transpose(0, 2, 1, 3)).reshape(CH, -1)

    colt_row = np.empty(NB * 3 * BLK, dtype=np.float32)
    for b in range(NB):
        colt_row[b * 3 * BLK: b * 3 * BLK + BLK] = ccol[b * BLK:(b + 1) * BLK]
        colt_row[b * 3 * BLK + BLK: b * 3 * BLK + 2 * BLK] = \
            cyinv[b * BLK:(b + 1) * BLK]
        colt_row[b * 3 * BLK + 2 * BLK: b * 3 * BLK + 3 * BLK] = \
            rpn[b * BLK:(b + 1) * BLK]
    colt_arr = np.ascontiguousarray(
        np.broadcast_to(colt_row[None, :], (CH, NB * 3 * BLK)))

    mk_arr = np.ascontiguousarray(tp)          # [49, NR] (quirk: moving = tp)

    in_maps = []
    for c in range(N_CORES):
        rows = slice(c * TPC, (c + 1) * TPC)
        xk_arr = np.ascontiguousarray(
            xgb[:, :, rows].reshape(CH, K * TPC))
        rowt_arr = np.zeros((CH, 4 * MT), dtype=np.float32)
        for m in range(MT):
            rsl = slice(c * TPC + m * 128, c * TPC + (m + 1) * 128)
            rowt_arr[:, 4 * m] = a_row[rsl].astype(np.float32)
            rowt_arr[:, 4 * m + 1] = rxinv[rsl].astype(np.float32)
            rowt_arr[:, 4 * m + 2] = tpn[rsl].astype(np.float32)
        # quirk: stationary = rp sliced to core rows; moving = tp (full)
        mpk_arr = np.concatenate([rp[:, rows], mk_arr], axis=1)
        mpk_arr = np.ascontiguousarray(mpk_arr)
        in_maps.append({
            "xk": xk_arr,
            "yk": yk_arr,
            "rowt": rowt_arr,
            "colt": colt_arr,
            "mpk": mpk_arr,
        })
    return in_maps


def finish(stats_list):
    """stats_list: per-core [128, 2*MT] f32 -> scalar loss."""
    losses = np.empty(NT, dtype=np.float64)
    for c, st in enumerate(stats_list):
        st = np.asarray(st, dtype=np.float64)
        for m in range(MT):
            dmin = st[:, 2 * m]
            sumw = st[:, 2 * m + 1]
            rec = 1.0 / (dmin + EPS)
            losses[c * TPC + m * 128: c * TPC + (m + 1) * 128] = (
                np.log(sumw) - 2.0 * (1.0 - dmin * rec))
    return np.float32(losses.mean())


def kernel(target_features, refer_features, mask, target_field, refer_field):
    from concourse.bass_utils import run_bass_kernel_spmd

    nc, _ = build_program()
    in_maps = host_prepare(target_features, refer_features, mask,
                           target_field, refer_field)
    res = run_bass_kernel_spmd(nc, in_maps, core_ids=list(range(N_CORES)))
    stats_list = [r["stats"] for r in res.results]
    return finish(stats_list)


if __name__ == "__main__":
    # smoke test with random inputs
    rng = np.random.default_rng(0)
    inputs = {
        "target_features": rng.random((1, 128, 256, 256), dtype=np.float32),
        "refer_features": rng.random((1, 128, 256, 256), dtype=np.float32),
        "mask": rng.random((1, 1, 256, 256), dtype=np.float32),
        "target_field": (rng.random((1, 64, 64, 2), dtype=np.float32) * 2 - 1),
        "refer_field": (rng.random((1, 64, 64, 2), dtype=np.float32) * 2 - 1),
    }
    out = kernel(**inputs)
    print("kernel loss:", out)


# revision 16
# speedup vs baseline: 2278238.2619x; 2278238.2619x over previous
"""Trainium2 Bass kernel for nn_GuidedCorrespondenceLoss.

Strategy (8 NeuronCores, SPMD):
  - Host: nearest-neighbor field -> patch indices; gather 7x7x128 patch
    columns into channel-major bf16 matrices; compute Gram-matrix correction
    vectors (means/norms/cross terms) so the device works on RAW gathered
    patches (no centering/normalization pass on device).
  - Device (per core, target rows sharded 512/core): S = Xg^T Yg via bf16
    matmuls (K=6272 on partitions, 49 k-tiles), mask-patch L2 matmul (K=49,
    f32), fused distance combine on Vector/Scalar engines, then row-min and
    exp-sum row stats (relaxed contextual loss).
  - Host: loss_i = log(sumw_i) - 2*(1 - dmin_i/(dmin_i+EPS)); mean over 4096.

The reference l2_distance has an index quirk (verified numerically):
  d_prog[i,j] = clip(||tp_i||^2 + ||rp_j||^2 - 2*tp[:,j].rp[:,i], 0)/49
so the device L2 matmul uses lhsT = rp[:, target_rows], rhs = tp[:, :].
"""
import numpy as np
import ml_dtypes

EPS = 2.220446049250313e-16
PATCH, STRIDE, SAMPLE, H_BW, PROG_W = 7, 3, 64, 0.5, 10.0
HO = (256 - PATCH) // STRIDE + 1     # 84

N_CORES = 8
NT = 4096            # total target rows
TPC = NT // N_CORES  # 512 target rows per core
MT = TPC // 128      # 4 m-tiles per core
NR = 4096            # refer columns
BLK = 256            # columns per block
NB = NR // BLK       # 16 blocks
K = PATCH * PATCH    # 49 contraction tiles of 128 channels
CH = 128

_PROGRAM_CACHE = {}


def _field_to_idx(field):
    g = np.asarray(field)[0].reshape(-1, 2).astype(np.float32)
    gx = (g[:, 0] + np.float32(1.0)) * np.float32(0.5) * np.float32(HO - 1)
    gy = (g[:, 1] + np.float32(1.0)) * np.float32(0.5) * np.float32(HO - 1)
    ix = np.clip(np.round(gx), 0, HO - 1).astype(np.int64)
    iy = np.clip(np.round(gy), 0, HO - 1).astype(np.int64)
    return iy, ix


def _gather_cols(feat_chw, iy, ix):
    """feat [C,256,256] -> [C, 49, n]: out[c,k,j] = feat[c, 3*iy+kh, 3*ix+kw]."""
    iy3, ix3 = iy * STRIDE, ix * STRIDE
    kh = np.repeat(np.arange(PATCH), PATCH)
    kw = np.tile(np.arange(PATCH), PATCH)
    HH = iy3[None, :] + kh[:, None]
    WW = ix3[None, :] + kw[:, None]
    return feat_chw[:, HH, WW]


def _build_kernel_body(tc, aps, repeats=1):
    import concourse.bass as bass
    from concourse import mybir

    nc = tc.nc
    f32 = mybir.dt.float32
    bf16 = mybir.dt.bfloat16
    AF = mybir.ActivationFunctionType
    OP = mybir.AluOpType

    xk, yk, rowt, colt, mpk, stats = (
        aps["xk"], aps["yk"], aps["rowt"], aps["colt"],
        aps["mpk"], aps["stats"],
    )

    YH0 = 25 * BLK               # first-half cols (k = 0..24)
    YH1 = (K - 25) * BLK         # second-half cols (k = 25..48)

    with (
        tc.tile_pool(name="xpool", bufs=1) as xpool,
        tc.tile_pool(name="ypool", bufs=3) as ypool,
        tc.tile_pool(name="dpool", bufs=1) as dpool,
        tc.tile_pool(name="cpool", bufs=2) as cpool,
        tc.tile_pool(name="mpool", bufs=1) as mpool,
        tc.tile_pool(name="tpool", bufs=2) as tpool,
        tc.tile_pool(name="spool", bufs=8) as spool,
        tc.tile_pool(name="psS", bufs=4, space="PSUM") as psS,
        tc.tile_pool(name="psP", bufs=2, space="PSUM") as psP,
    ):
        # ---- resident loads ------------------------------------------------
        x_t = xpool.tile([CH, K * TPC], bf16, name="x_t")
        nc.sync.dma_start(x_t[:], xk[:])

        rowt_t = xpool.tile([CH, 4 * MT], f32, name="rowt_t")
        nc.sync.dma_start(rowt_t[:], rowt[:])

        cst = xpool.tile([CH, 2], f32, name="cst")
        nc.vector.memset(cst[:, 0:1], 0.5)
        nc.vector.memset(cst[:, 1:2], 2.0)

        # mq (rp[:,rows], TPC cols) and mk (tp, NR cols) share one DMA/tensor
        mpk_t = mpool.tile([CH, TPC + NR], f32, name="mpk_t")
        nc.sync.dma_start(mpk_t[:K, :], mpk[:])
        mq_t = mpk_t[:, :TPC]
        mk_t = mpk_t[:, TPC:]

        for rep in range(repeats):
            _emit_rep(tc, nc, aps, locals(), rep)


def _emit_rep(tc, nc, aps, env, rep):
    import concourse.bass as bass
    from concourse import mybir

    f32 = mybir.dt.float32
    bf16 = mybir.dt.bfloat16
    AF = mybir.ActivationFunctionType
    OP = mybir.AluOpType

    yk, colt, stats = aps["yk"], aps["colt"], aps["stats"]
    x_t, rowt_t, cst, mq_t, mk_t = (
        env["x_t"], env["rowt_t"], env["cst"], env["mq_t"], env["mk_t"])
    xpool, ypool, dpool, cpool, tpool, spool, psS, psP = (
        env["xpool"], env["ypool"], env["dpool"], env["cpool"],
        env["tpool"], env["spool"], env["psS"], env["psP"])
    YH0 = env["YH0"]
    YH1 = env["YH1"]

    if True:
        d_t = dpool.tile([CH, MT * NR], f32, name=f"d_t_r{rep}", tag="d")

        # ---- main loop -----------------------------------------------------
        for b in range(NB):
            y0 = ypool.tile([CH, YH0], bf16, name=f"y0_r{rep}_{b}", tag="y")
            nc.sync.dma_start(y0[:], yk[:, b * K * BLK: b * K * BLK + YH0])
            y1 = ypool.tile([CH, YH0], bf16, name=f"y1_r{rep}_{b}", tag="y")
            nc.sync.dma_start(
                y1[:, :YH1], yk[:, b * K * BLK + YH0: (b + 1) * K * BLK])

            colt_t = cpool.tile([CH, 3 * BLK], f32, name=f"colt_r{rep}_{b}", tag="c")
            nc.sync.dma_start(colt_t[:], colt[:, b * 3 * BLK: (b + 1) * 3 * BLK])

            mkb = mk_t[:K, b * BLK: (b + 1) * BLK]

            for m in range(MT):
                pp = psP.tile([CH, BLK], f32, name=f"pp_r{rep}_{b}_{m}", tag="pp")
                nc.tensor.matmul(
                    pp[:],
                    lhsT=mq_t[:K, m * 128: (m + 1) * 128],
                    rhs=mkb,
                    start=True, stop=True,
                )
                ps = psS.tile([CH, BLK], f32, name=f"ps_r{rep}_{b}_{m}", tag="ps")
                for k in range(K):
                    if k < 25:
                        rhs = y0[:, k * BLK: (k + 1) * BLK]
                    else:
                        rhs = y1[:, (k - 25) * BLK: (k - 24) * BLK]
                    nc.tensor.matmul(
                        ps[:],
                        lhsT=x_t[:, k * TPC + m * 128: k * TPC + (m + 1) * 128],
                        rhs=rhs,
                        start=(k == 0),
                        stop=(k == K - 1),
                    )

                # sim = (S + a_p + ccol_j) * rxinv_p * cyinv_j
                t2 = tpool.tile([CH, BLK], f32, name=f"t2_r{rep}_{b}_{m}", tag="t2")
                nc.vector.scalar_tensor_tensor(
                    t2[:], in0=ps[:], scalar=rowt_t[:, 4 * m: 4 * m + 1],
                    in1=colt_t[:, 0:BLK], op0=OP.add, op1=OP.add)
                t4 = tpool.tile([CH, BLK], f32, name=f"t4_r{rep}_{b}_{m}", tag="t4")
                nc.vector.scalar_tensor_tensor(
                    t4[:], in0=t2[:], scalar=rowt_t[:, 4 * m + 1: 4 * m + 2],
                    in1=colt_t[:, BLK: 2 * BLK], op0=OP.mult, op1=OP.mult)
                # d_cos = relu(0.5 - 0.5*sim)
                dcos = tpool.tile([CH, BLK], f32, name=f"dcos_r{rep}_{b}_{m}", tag="dcos")
                nc.scalar.activation(dcos[:], t4[:], AF.Relu,
                                     bias=cst[:, 0:1], scale=-0.5)
                # v = relu(tpn_p + rpn_j - 2*P)
                u = tpool.tile([CH, BLK], f32, name=f"u_r{rep}_{b}_{m}", tag="u")
                nc.vector.scalar_tensor_tensor(
                    u[:], in0=pp[:], scalar=-2.0,
                    in1=colt_t[:, 2 * BLK: 3 * BLK], op0=OP.mult, op1=OP.add)
                v = tpool.tile([CH, BLK], f32, name=f"v_r{rep}_{b}_{m}", tag="v")
                nc.scalar.activation(
                    v[:], u[:], AF.Relu,
                    bias=rowt_t[:, 4 * m + 2: 4 * m + 3], scale=1.0)
                # d = dcos + (10/49) * v   -> f32 store
                nc.vector.scalar_tensor_tensor(
                    d_t[:, m * NR + b * BLK: m * NR + (b + 1) * BLK],
                    in0=v[:], scalar=float(PROG_W / K), in1=dcos[:],
                    op0=OP.mult, op1=OP.add)

        # ---- row stats -----------------------------------------------------
        souts = xpool.tile([CH, 2 * MT], f32, name=f"souts_r{rep}", tag="so")
        for m in range(MT):
            dm = d_t[:, m * NR: (m + 1) * NR]
            dmin = spool.tile([CH, 1], f32, name=f"dmin_r{rep}_{m}", tag="st")
            nc.vector.tensor_reduce(dmin[:], dm, axis=mybir.AxisListType.X,
                                    op=OP.min)
            dme = spool.tile([CH, 1], f32, name=f"dme_r{rep}_{m}", tag="st")
            nc.vector.tensor_scalar_add(dme[:], dmin[:], float(EPS))
            rec = spool.tile([CH, 1], f32, name=f"rec_r{rep}_{m}", tag="st")
            nc.vector.reciprocal(rec[:], dme[:])
            scl = spool.tile([CH, 1], f32, name=f"scl_r{rep}_{m}", tag="st")
            nc.vector.tensor_scalar_mul(scl[:], rec[:], -2.0)
            # w = exp(2 - 2*d*rec); sumw = sum_j w
            wtmp = ypool.tile([CH, NR], bf16, name=f"wtmp_r{rep}_{m}", tag="y")
            sumw = spool.tile([CH, 1], f32, name=f"sumw_r{rep}_{m}", tag="st")
            nc.scalar.activation(wtmp[:], dm, AF.Exp,
                                 bias=cst[:, 1:2], scale=scl[:, 0:1],
                                 accum_out=sumw[:])
            nc.vector.tensor_copy(souts[:, 2 * m: 2 * m + 1], dmin[:])
            nc.vector.tensor_copy(souts[:, 2 * m + 1: 2 * m + 2], sumw[:])

        nc.sync.dma_start(stats[:], souts[:])


def build_program(repeats=1):
    """Build the SPMD Bass program once. Returns (nc, names)."""
    key = ("prog", repeats)
    if key in _PROGRAM_CACHE:
        return _PROGRAM_CACHE[key]

    import concourse.tile as tile
    from concourse import bacc, mybir

    f32 = mybir.dt.float32
    bf16 = mybir.dt.bfloat16

    nc = bacc.Bacc("TRN2", target_bir_lowering=False, debug=False,
                   enable_asserts=False, num_devices=N_CORES)
    aps = {
        "xk": nc.dram_tensor("xk", [CH, K * TPC], bf16,
                             kind="ExternalInput").ap(),
        "yk": nc.dram_tensor("yk", [CH, NB * K * BLK], bf16,
                             kind="ExternalInput").ap(),
        "rowt": nc.dram_tensor("rowt", [CH, 4 * MT], f32,
                               kind="ExternalInput").ap(),
        "colt": nc.dram_tensor("colt", [CH, NB * 3 * BLK], f32,
                               kind="ExternalInput").ap(),
        "mpk": nc.dram_tensor("mpk", [K, TPC + NR], f32,
                              kind="ExternalInput").ap(),
        "stats": nc.dram_tensor("stats", [CH, 2 * MT], f32,
                                kind="ExternalOutput").ap(),
    }
    with tile.TileContext(nc) as tc:
        _build_kernel_body(tc, aps, repeats=repeats)
    nc.compile()

    _PROGRAM_CACHE[key] = (nc, aps)
    return nc, aps


def host_prepare(target_features, refer_features, mask, target_field,
                 refer_field):
    """Compute everything the device needs. Returns (in_maps, meta)."""
    tgt = np.asarray(target_features)[0]
    ref = np.asarray(refer_features)[0]
    msk = np.asarray(mask)[0, 0]
    t_iy, t_ix = _field_to_idx(target_field)
    r_iy, r_ix = _field_to_idx(refer_field)

    tgt32 = tgt.astype(ml_dtypes.bfloat16).astype(np.float32)
    ref32 = ref.astype(ml_dtypes.bfloat16).astype(np.float32)

    xg = _gather_cols(tgt32, t_iy, t_ix)       # [128, 49, 4096] f32
    yg = _gather_cols(ref32, r_iy, r_ix)

    xg2 = xg.reshape(CH * K, NT).astype(np.float64)
    yg2 = yg.reshape(CH * K, NR)
    yg64 = yg2.astype(np.float64)
    mu = yg64.mean(axis=1)
    crx = mu @ xg2
    cry = mu @ yg64
    musq = float(mu @ mu)
    xnormsq = np.einsum('ij,ij->j', xg2, xg2) - 2 * crx + musq
    ynormsq = np.einsum('ij,ij->j', yg64, yg64) - 2 * cry + musq
    rxinv = 1.0 / (np.sqrt(np.maximum(xnormsq, 0)) + EPS)
    cyinv = 1.0 / (np.sqrt(np.maximum(ynormsq, 0)) + EPS)
    a_row = musq - crx
    ccol = -cry

    tp = _gather_cols(msk[None], t_iy, t_ix)[0].astype(np.float32)  # [49, NT]
    rp = _gather_cols(msk[None], r_iy, r_ix)[0].astype(np.float32)  # [49, NR]
    tpn = (tp.astype(np.float64) ** 2).sum(0)
    rpn = (rp.astype(np.float64) ** 2).sum(0)

    # device input assembly
    xgb = xg.astype(ml_dtypes.bfloat16)        # [128, 49, 4096]
    ygb = yg.astype(ml_dtypes.bfloat16)
    # yk layout: col = b*K*BLK + k*BLK + j
    yk_arr = np.ascontiguousarray(
        ygb.reshape(CH, K, NB, BLK).transpose(0, 2, 1, 3)).reshape(CH, -1)

    colt_row = np.empty(NB * 3 * BLK, dtype=np.float32)
    for b in range(NB):
        colt_row[b * 3 * BLK: b * 3 * BLK + BLK] = ccol[b * BLK:(b + 1) * BLK]
        colt_row[b * 3 * BLK + BLK: b * 3 * BLK + 2 * BLK] = \
            cyinv[b * BLK:(b + 1) * BLK]
        colt_row[b * 3 * BLK + 2 * BLK: b * 3 * BLK + 3 * BLK] = \
            rpn[b * BLK:(b + 1) * BLK]
    colt_arr = np.ascontiguousarray(
        np.broadcast_to(colt_row[None, :], (CH, NB * 3 * BLK)))

    mk_arr = np.ascontiguousarray(tp)          # [49, NR] (quirk: moving = tp)

    in_maps = []
    for c in range(N_CORES):
        rows = slice(c * TPC, (c + 1) * TPC)
        xk_arr = np.ascontiguousarray(
            xgb[:, :, rows].reshape(CH, K * TPC))
        rowt_arr = np.zeros((CH, 4 * MT), dtype=np.float32)
        for m in range(MT):
            rsl = slice(c * TPC + m * 128, c * TPC + (m + 1) * 128)
            rowt_arr[:, 4 * m] = a_row[rsl].astype(np.float32)
            rowt_arr[:, 4 * m + 1] = rxinv[rsl].astype(np.float32)
            rowt_arr[:, 4 * m + 2] = tpn[rsl].astype(np.float32)
        # quirk: stationary = rp sliced to core rows; moving = tp (full)
        mpk_arr = np.concatenate([rp[:, rows], mk_arr], axis=1)
        mpk_arr = np.ascontiguousarray(mpk_arr)
        in_maps.append({
            "xk": xk_arr,
            "yk": yk_arr,
            "rowt": rowt_arr,
            "colt": colt_arr,
            "mpk": mpk_arr,
        })
    return in_maps


def finish(stats_list):
    """stats_list: per-core [128, 2*MT] f32 -> scalar loss."""
    losses = np.empty(NT, dtype=np.float64)
    for c, st in enumerate(stats_list):
        st = np.asarray(st, dtype=np.float64)
        for m in range(MT):
            dmin = st[:, 2 * m]
            sumw = st[:, 2 * m + 1]
            rec = 1.0 / (dmin + EPS)
            losses[c * TPC + m * 128: c * TPC + (m + 1) * 128] = (
                np.log(sumw) - 2.0 * (1.0 - dmin * rec))
    return np.float32(losses.mean())


def kernel(target_features, refer_features, mask, target_field, refer_field):
    from concourse.bass_utils import run_bass_kernel_spmd

    nc, _ = build_program()
    in_maps = host_prepare(target_features, refer_features, mask,
                           target_field, refer_field)
    res = run_bass_kernel_spmd(nc, in_maps, core_ids=list(range(N_CORES)))
    stats_list = [r["stats"] for r in res.results]
    return finish(stats_list)


if __name__ == "__main__":
    # smoke test with random inputs
    rng = np.random.default_rng(0)
    inputs = {
        "target_features": rng.random((1, 128, 256, 256), dtype=np.float32),
        "refer_features": rng.random((1, 128, 256, 256), dtype=np.float32),
        "mask": rng.random((1, 1, 256, 256), dtype=np.float32),
        "target_field": (rng.random((1, 64, 64, 2), dtype=np.float32) * 2 - 1),
        "refer_field": (rng.random((1, 64, 64, 2), dtype=np.float32) * 2 - 1),
    }
    out = kernel(**inputs)
    print("kernel loss:", out)
